# revision 3
# baseline (speedup 1.0000x reference)
"""BiRNN (nn_BiRNNScratch) Trainium2 Bass kernel.

Strategy (8 NeuronCores):
  - cores 0-3: forward direction, batch quarters 0-3 (16 rows each)
  - cores 4-7: backward direction (host pre-reverses time), batch quarters 0-3
  Every core runs the SAME single-direction RNN program (SPMD).

Per-core program (T=512 steps, B=16 local batch, NH=NI=512):
  - state kept transposed: hT [NH(4x128 part-tiles), B] in bf16
  - xw_t = x_t @ W_xh + b precomputed per 32-step block straight into PSUM
    (4 banks, one per NH out-tile; bias added via a K=1 matmul with ones)
  - recurrence accumulates h_{t-1} @ W_hh on top of the xw PSUM via 16
    matmuls/step (W_hh tiles as stationary operand, bf16 => fast weight load)
  - one Tanh activation per step reads the 4 bank regions and writes the
    bf16 transposed state history, which doubles as the next step's rhs
  - history blocks are DMAd out in [NH, T, B] layout; host transposes back.

Host side does the sharding glue: batch split, time reversal for the
backward direction, x transpose to [NI, T, B], weight tiling, bf16 casts,
and the output gather/uncast/transpose.
"""

import os

# bass2jax needs the axon jax platform; guard against an inherited cpu pin.
_jp = os.environ.get("JAX_PLATFORMS")
if _jp is not None and "axon" not in _jp:
    del os.environ["JAX_PLATFORMS"]

import numpy as np
import ml_dtypes

import bass_rust
import concourse.bass as bass
import concourse.mybir as mybir
import concourse.tile as tile
from concourse.vector_clock import ScopedClock
from concourse.bass_utils import run_bass_kernel_spmd

T_FULL = 512
B_FULL = 64
NI = 512
NH = 512
N_CORES = 8
BL = 16            # batch rows per core
KC = NI // 128     # contraction chunks
MC = NH // 128     # output tiles
TC = 32            # time steps per block (one PSUM bank per out-tile)
BF16 = mybir.dt.bfloat16
F32 = mybir.dt.float32
NPBF16 = ml_dtypes.bfloat16

# ---------------------------------------------------------------------------
# Workaround: this toolchain's walrus rejects SP CTRL instructions carrying
# several sync waits.  Split the TileContext tail-drain's waits onto
# individual no-fuse NOPs (one wait each) and emit the drain bare.
_drain_patched = False


def _patch_drain():
    global _drain_patched
    if _drain_patched:
        return

    def _drain_and_barrier(self, tick_clock, wait_clock):
        nc = self.nc
        ticks = list(tick_clock.global_clock)
        for i, t in enumerate(ticks):
            if t:
                part = [0] * len(ticks)
                part[i] = t
                nopi = nc.sync.nop(nofuse=True)
                wait_clock.add_sem_waits(
                    nopi.ins, ScopedClock({None: bass_rust.VectorClock(part)})
                )
        nc.sync.drain()
        nc.all_engine_barrier()
        popped = nc._tile_sem_poison_stack.pop()
        assert popped is self._sem_poison
        nc.clear_and_free_semaphores(list(self.sems.allocated().values()))
        nc.all_engine_barrier()

    tile.TileContext._drain_and_barrier = _drain_and_barrier
    _drain_patched = True


def _split_waits(nc, limit: int = 1):
    """Walrus in this toolchain rejects instructions carrying more than one
    sync wait.  Move excess waits onto dedicated same-engine NoOps inserted
    immediately before the overloaded instruction (queue order preserves
    wait-before-execute semantics)."""
    for f in nc.m.functions:
        for bb in f.blocks:
            new_list = []
            changed = False
            for inst in bb.instructions:
                si = inst.sync_info
                waits = list(si.on_wait) if (si and si.on_wait) else []
                if len(waits) > limit:
                    excess, keep = waits[:-limit], waits[-limit:]
                    for j, w in enumerate(excess):
                        nop = mybir.InstNoOp(
                            name=f"{inst.name}-wsplit{j}", ins=[], outs=[],
                            engine=inst.engine,
                        )
                        nop.sync_info = bass_rust.SyncInfo(
                            on_wait=[w], on_update=[]
                        )
                        try:
                            nop.bass_nofuse = True
                        except Exception:
                            pass
                        new_list.append(nop)
                    si.on_wait = keep
                    changed = True
                new_list.append(inst)
            if changed:
                bb.instructions = new_list


# ---------------------------------------------------------------------------
def build_nc(t_steps: int = T_FULL):
    """Build the per-core single-direction RNN program."""
    _patch_drain()
    assert t_steps % TC == 0
    nblk = t_steps // TC
    nc = bass.Bass("TRN2", target_bir_lowering=False, debug=False)

    xt = nc.declare_dram_parameter("xt", [KC, 128, t_steps, BL], BF16, isOutput=False)
    whh = nc.declare_dram_parameter("whh", [128, KC, MC, 128], BF16, isOutput=False)
    wxh = nc.declare_dram_parameter("wxh", [128, KC, MC, 128], BF16, isOutput=False)
    bh = nc.declare_dram_parameter("bh", [1, MC, 128], BF16, isOutput=False)
    hs = nc.declare_dram_parameter("hs", [MC, 128, t_steps, BL], BF16, isOutput=True)

    xt_perm = xt[:, :, :, :].rearrange("k p t b -> p k t b")
    hs_perm = hs[:, :, :, :].rearrange("m p t b -> p m t b")

    with tile.TileContext(nc) as tc:
        with (
            tc.tile_pool(name="singles", bufs=1) as singles,
            tc.tile_pool(name="xtp", bufs=2) as xt_pool,
            tc.tile_pool(name="histp", bufs=3) as hist_pool,
            tc.tile_pool(name="psump", bufs=2, space="PSUM") as psum_pool,
        ):
            whh_sb = singles.tile([128, KC, MC, 128], BF16)
            nc.sync.dma_start(out=whh_sb, in_=whh[:, :, :, :])
            wxh_sb = singles.tile([128, KC, MC, 128], BF16)
            nc.sync.dma_start(out=wxh_sb, in_=wxh[:, :, :, :])
            bh_sb = singles.tile([1, MC, 128], BF16)
            nc.sync.dma_start(out=bh_sb, in_=bh[:, :, :])
            ones_sb = singles.tile([1, 512], BF16)
            nc.vector.memset(ones_sb, 1.0)

            state = {"prev_hist": None}

            def emit_rec(ps, blk):
                hist = hist_pool.tile([128, MC, TC, BL], BF16, tag="hist")
                for tl in range(TC):
                    gt = blk * TC + tl
                    if gt > 0:
                        src = state["prev_hist"] if tl == 0 else hist
                        pt = TC - 1 if tl == 0 else tl - 1
                        for m in range(MC):
                            for k in range(KC):
                                nc.tensor.matmul(
                                    ps[:, m, tl * BL:(tl + 1) * BL],
                                    lhsT=whh_sb[:, k, m, :],
                                    rhs=src[:, k, pt, :],
                                    start=False,
                                    stop=(k == KC - 1),
                                    skip_group_check=True,
                                )
                    nc.scalar.activation(
                        hist[:, :, tl, :],
                        ps[:, :, tl * BL:(tl + 1) * BL],
                        mybir.ActivationFunctionType.Tanh,
                    )
                nc.sync.dma_start(
                    out=hs_perm[:, :, blk * TC:(blk + 1) * TC, :], in_=hist
                )
                state["prev_hist"] = hist

            pending = None
            for blk in range(nblk):
                xt_sb = xt_pool.tile([128, KC, TC * BL], BF16, tag="xt")
                nc.sync.dma_start(
                    out=xt_sb, in_=xt_perm[:, :, blk * TC:(blk + 1) * TC, :]
                )
                ps = psum_pool.tile([128, MC, 512], F32, tag="ps")
                for m in range(MC):
                    for k in range(KC):
                        nc.tensor.matmul(
                            ps[:, m, :],
                            lhsT=wxh_sb[:, k, m, :],
                            rhs=xt_sb[:, k, :],
                            start=(k == 0),
                            stop=False,
                            skip_group_check=True,
                        )
                    nc.tensor.matmul(
                        ps[:, m, :],
                        lhsT=bh_sb[:, m, :],
                        rhs=ones_sb[:, :],
                        start=False,
                        stop=True,
                        skip_group_check=True,
                    )
                if pending is not None:
                    emit_rec(*pending)
                pending = (ps, blk)
            emit_rec(*pending)

    _split_waits(nc)
    return nc


# ---------------------------------------------------------------------------
def _tile_weight(w: np.ndarray) -> np.ndarray:
    # [NI, NH] -> [128, KC, MC, 128]: tiles[p, k, m, c] = w[k*128+p, m*128+c]
    return np.ascontiguousarray(
        w.reshape(KC, 128, MC, 128).transpose(1, 0, 2, 3)
    ).astype(NPBF16)


def make_in_maps(inputs, W_xh_f, W_hh_f, b_h_f, W_xh_b, W_hh_b, b_h_b,
                 t_steps: int = T_FULL):
    inputs = np.asarray(inputs, dtype=np.float32)
    wf = {
        "whh": _tile_weight(np.asarray(W_hh_f, np.float32)),
        "wxh": _tile_weight(np.asarray(W_xh_f, np.float32)),
        "bh": np.asarray(b_h_f, np.float32).reshape(1, MC, 128).astype(NPBF16),
    }
    wb = {
        "whh": _tile_weight(np.asarray(W_hh_b, np.float32)),
        "wxh": _tile_weight(np.asarray(W_xh_b, np.float32)),
        "bh": np.asarray(b_h_b, np.float32).reshape(1, MC, 128).astype(NPBF16),
    }
    in_maps = []
    for c in range(N_CORES):
        fwd = c < 4
        q = c % 4
        x = inputs[:, q * BL:(q + 1) * BL, :]
        if not fwd:
            x = x[::-1]
        # [T, BL, NI] -> [KC, 128, T, BL]
        xt = np.ascontiguousarray(x.transpose(2, 0, 1)).reshape(KC, 128, t_steps, BL)
        m = {"xt": xt.astype(NPBF16)}
        m.update(wf if fwd else wb)
        in_maps.append(m)
    return in_maps


def assemble_outputs(results, t_steps: int = T_FULL):
    outputs = np.empty((t_steps, B_FULL, 2 * NH), dtype=np.float32)
    f_H = np.empty((B_FULL, NH), dtype=np.float32)
    b_H = np.empty((B_FULL, NH), dtype=np.float32)
    for c in range(N_CORES):
        fwd = c < 4
        q = c % 4
        bsl = slice(q * BL, (q + 1) * BL)
        hsb = np.asarray(results[c]["hs"])  # [MC, 128, T, BL] bf16
        # -> [T, BL, NH] fp32
        h = hsb.reshape(NH, t_steps, BL).transpose(1, 2, 0).astype(np.float32)
        if fwd:
            outputs[:, bsl, :NH] = h
            f_H[bsl] = h[-1]
        else:
            outputs[:, bsl, NH:] = h[::-1]
            b_H[bsl] = h[-1]
    return outputs, f_H, b_H


_nc_cache = {}


def get_nc(t_steps: int = T_FULL):
    if t_steps not in _nc_cache:
        _nc_cache[t_steps] = build_nc(t_steps)
    return _nc_cache[t_steps]


def kernel(inputs, W_xh_f, W_hh_f, b_h_f, W_xh_b, W_hh_b, b_h_b):
    nc = get_nc(T_FULL)
    in_maps = make_in_maps(
        inputs, W_xh_f, W_hh_f, b_h_f, W_xh_b, W_hh_b, b_h_b, T_FULL
    )
    res = run_bass_kernel_spmd(nc, in_maps, list(range(N_CORES)))
    return assemble_outputs(res.results, T_FULL)


# revision 11
# speedup vs baseline: 151.4399x; 151.4399x over previous
"""BiRNN (nn_BiRNNScratch) Trainium2 Bass kernel.

Strategy (8 NeuronCores):
  - cores 0-3: forward direction, batch quarters 0-3 (16 rows each)
  - cores 4-7: backward direction (host pre-reverses time), batch quarters 0-3
  Every core runs the SAME single-direction RNN program (SPMD).

Per-core program (T=512 steps, B=16 local batch, NH=NI=512):
  - state kept transposed: hT [NH(4x128 part-tiles), B] in bf16
  - xw_t = x_t @ W_xh + b precomputed per 32-step block straight into PSUM
    (4 banks, one per NH out-tile; bias added via a K=1 matmul with ones)
  - recurrence accumulates h_{t-1} @ W_hh on top of the xw PSUM via 16
    matmuls/step (W_hh tiles as stationary operand, bf16 => fast weight load)
  - one Tanh activation per step reads the 4 bank regions and writes the
    bf16 transposed state history, which doubles as the next step's rhs
  - history blocks are DMAd out in [NH, T, B] layout; host transposes back.

Host side does the sharding glue: batch split, time reversal for the
backward direction, x transpose to [NI, T, B], weight tiling, bf16 casts,
and the output gather/uncast/transpose.
"""

import os

# bass2jax needs the axon jax platform; guard against an inherited cpu pin.
_jp = os.environ.get("JAX_PLATFORMS")
if _jp is not None and "axon" not in _jp:
    del os.environ["JAX_PLATFORMS"]

import numpy as np
import ml_dtypes

import bass_rust
import concourse.bass as bass
import concourse.mybir as mybir
import concourse.tile as tile
from concourse.vector_clock import ScopedClock
from concourse.bass_utils import run_bass_kernel_spmd

T_FULL = 512
B_FULL = 64
NI = 512
NH = 512
N_CORES = 8
BL = 16            # batch rows per core
KC = NI // 128     # contraction chunks
MC = NH // 128     # output tiles
TC = 32            # time steps per block (one PSUM bank per out-tile)
BF16 = mybir.dt.bfloat16
F32 = mybir.dt.float32
NPBF16 = ml_dtypes.bfloat16

# ---------------------------------------------------------------------------
# Workaround: this toolchain's walrus rejects SP CTRL instructions carrying
# several sync waits.  Split the TileContext tail-drain's waits onto
# individual no-fuse NOPs (one wait each) and emit the drain bare.
_drain_patched = False


def _patch_drain():
    global _drain_patched
    if _drain_patched:
        return

    def _drain_and_barrier(self, tick_clock, wait_clock):
        nc = self.nc
        ticks = list(tick_clock.global_clock)
        for i, t in enumerate(ticks):
            if t:
                part = [0] * len(ticks)
                part[i] = t
                nopi = nc.sync.nop(nofuse=True)
                wait_clock.add_sem_waits(
                    nopi.ins, ScopedClock({None: bass_rust.VectorClock(part)})
                )
        nc.sync.drain()
        nc.all_engine_barrier()
        popped = nc._tile_sem_poison_stack.pop()
        assert popped is self._sem_poison
        nc.clear_and_free_semaphores(list(self.sems.allocated().values()))
        nc.all_engine_barrier()

    tile.TileContext._drain_and_barrier = _drain_and_barrier
    _drain_patched = True


def _split_waits(nc, limit: int = 1):
    """Walrus in this toolchain rejects instructions carrying more than one
    sync wait.  Move excess waits onto dedicated same-engine NoOps inserted
    immediately before the overloaded instruction (queue order preserves
    wait-before-execute semantics)."""
    for f in nc.m.functions:
        for bb in f.blocks:
            new_list = []
            changed = False
            for inst in bb.instructions:
                si = inst.sync_info
                waits = list(si.on_wait) if (si and si.on_wait) else []
                if len(waits) > limit:
                    excess, keep = waits[:-limit], waits[-limit:]
                    for j, w in enumerate(excess):
                        nop = mybir.InstNoOp(
                            name=f"{inst.name}-wsplit{j}", ins=[], outs=[],
                            engine=inst.engine,
                        )
                        nop.sync_info = bass_rust.SyncInfo(
                            on_wait=[w], on_update=[]
                        )
                        try:
                            nop.bass_nofuse = True
                        except Exception:
                            pass
                        new_list.append(nop)
                    si.on_wait = keep
                    changed = True
                new_list.append(inst)
            if changed:
                bb.instructions = new_list


# ---------------------------------------------------------------------------
def build_nc(t_steps: int = T_FULL, repeats: int = 1):
    """Build the per-core single-direction RNN program.

    repeats > 1 re-runs the whole pass (each pass restarts from h0=0, so
    outputs are identical) — used to measure device time by slope."""
    _patch_drain()
    assert t_steps % TC == 0
    nblk = t_steps // TC
    nc = bass.Bass("TRN2", target_bir_lowering=False, debug=False)

    xt = nc.declare_dram_parameter("xt", [KC, 128, t_steps, BL], BF16, isOutput=False)
    whh = nc.declare_dram_parameter("whh", [128, KC, MC, 128], BF16, isOutput=False)
    wxh = nc.declare_dram_parameter("wxh", [128, KC, MC, 128], BF16, isOutput=False)
    bh = nc.declare_dram_parameter("bh", [1, MC, 128], BF16, isOutput=False)
    hs = nc.declare_dram_parameter("hs", [MC, 128, t_steps, BL], BF16, isOutput=True)

    xt_perm = xt[:, :, :, :].rearrange("k p t b -> p k t b")
    hs_perm = hs[:, :, :, :].rearrange("m p t b -> p m t b")

    with tile.TileContext(nc) as tc:
        with (
            tc.tile_pool(name="singles", bufs=1) as singles,
            tc.tile_pool(name="xtp", bufs=2) as xt_pool,
            tc.tile_pool(name="histp", bufs=3) as hist_pool,
            tc.tile_pool(name="psump", bufs=2, space="PSUM") as psum_pool,
        ):
            whh_sb = singles.tile([128, KC, MC, 128], BF16)
            nc.sync.dma_start(out=whh_sb, in_=whh[:, :, :, :])
            wxh_sb = singles.tile([128, KC, MC, 128], BF16)
            nc.sync.dma_start(out=wxh_sb, in_=wxh[:, :, :, :])
            bh_sb = singles.tile([1, MC, 128], BF16)
            nc.sync.dma_start(out=bh_sb, in_=bh[:, :, :])
            ones_sb = singles.tile([1, 512], BF16)
            nc.vector.memset(ones_sb, 1.0)

            state = {"prev_hist": None}

            def emit_rec(ps, blk, pass_first):
                hist = hist_pool.tile([128, MC, TC, BL], BF16, tag="hist")
                for tl in range(TC):
                    if not (pass_first and tl == 0):
                        src = state["prev_hist"] if tl == 0 else hist
                        pt = TC - 1 if tl == 0 else tl - 1
                        for m in range(MC):
                            for k in range(KC):
                                nc.tensor.matmul(
                                    ps[:, m, tl * BL:(tl + 1) * BL],
                                    lhsT=whh_sb[:, k, m, :],
                                    rhs=src[:, k, pt, :],
                                    start=False,
                                    stop=(k == KC - 1),
                                    skip_group_check=True,
                                )
                    nc.scalar.activation(
                        hist[:, :, tl, :],
                        ps[:, :, tl * BL:(tl + 1) * BL],
                        mybir.ActivationFunctionType.Tanh,
                    )
                nc.sync.dma_start(
                    out=hs_perm[:, :, blk * TC:(blk + 1) * TC, :], in_=hist
                )
                state["prev_hist"] = hist

            pending = None
            for rep in range(repeats):
                for blk in range(nblk):
                    xt_sb = xt_pool.tile([128, KC, TC * BL], BF16, tag="xt")
                    nc.sync.dma_start(
                        out=xt_sb, in_=xt_perm[:, :, blk * TC:(blk + 1) * TC, :]
                    )
                    ps = psum_pool.tile([128, MC, 512], F32, tag="ps")
                    for m in range(MC):
                        for k in range(KC):
                            nc.tensor.matmul(
                                ps[:, m, :],
                                lhsT=wxh_sb[:, k, m, :],
                                rhs=xt_sb[:, k, :],
                                start=(k == 0),
                                stop=False,
                                skip_group_check=True,
                            )
                        nc.tensor.matmul(
                            ps[:, m, :],
                            lhsT=bh_sb[:, m, :],
                            rhs=ones_sb[:, :],
                            start=False,
                            stop=True,
                            skip_group_check=True,
                        )
                    if pending is not None:
                        emit_rec(*pending)
                    pending = (ps, blk, blk == 0)
                emit_rec(*pending)
                pending = None

    _split_waits(nc)
    return nc


# ---------------------------------------------------------------------------
def _tile_weight(w: np.ndarray) -> np.ndarray:
    # [NI, NH] -> [128, KC, MC, 128]: tiles[p, k, m, c] = w[k*128+p, m*128+c]
    return np.ascontiguousarray(
        w.reshape(KC, 128, MC, 128).transpose(1, 0, 2, 3)
    ).astype(NPBF16)


def make_in_maps(inputs, W_xh_f, W_hh_f, b_h_f, W_xh_b, W_hh_b, b_h_b,
                 t_steps: int = T_FULL):
    inputs = np.asarray(inputs, dtype=np.float32)
    wf = {
        "whh": _tile_weight(np.asarray(W_hh_f, np.float32)),
        "wxh": _tile_weight(np.asarray(W_xh_f, np.float32)),
        "bh": np.asarray(b_h_f, np.float32).reshape(1, MC, 128).astype(NPBF16),
    }
    wb = {
        "whh": _tile_weight(np.asarray(W_hh_b, np.float32)),
        "wxh": _tile_weight(np.asarray(W_xh_b, np.float32)),
        "bh": np.asarray(b_h_b, np.float32).reshape(1, MC, 128).astype(NPBF16),
    }
    in_maps = []
    for c in range(N_CORES):
        fwd = c < 4
        q = c % 4
        x = inputs[:, q * BL:(q + 1) * BL, :]
        if not fwd:
            x = x[::-1]
        # [T, BL, NI] -> [KC, 128, T, BL]
        xt = np.ascontiguousarray(x.transpose(2, 0, 1)).reshape(KC, 128, t_steps, BL)
        if USE_V2:
            xt = _parity_group_xt(xt, t_steps)
        m = {"xt": xt.astype(NPBF16)}
        m.update(wf if fwd else wb)
        in_maps.append(m)
    return in_maps


def assemble_outputs(results, t_steps: int = T_FULL):
    outputs = np.empty((t_steps, B_FULL, 2 * NH), dtype=np.float32)
    f_H = np.empty((B_FULL, NH), dtype=np.float32)
    b_H = np.empty((B_FULL, NH), dtype=np.float32)
    for c in range(N_CORES):
        fwd = c < 4
        q = c % 4
        bsl = slice(q * BL, (q + 1) * BL)
        hsb = np.asarray(results[c]["hs"])  # [MC, 128, T, BL] bf16
        # -> [T, BL, NH] fp32
        h = hsb.reshape(NH, t_steps, BL).transpose(1, 2, 0).astype(np.float32)
        if fwd:
            outputs[:, bsl, :NH] = h
            f_H[bsl] = h[-1]
        else:
            outputs[:, bsl, NH:] = h[::-1]
            b_H[bsl] = h[-1]
    return outputs, f_H, b_H


# ---------------------------------------------------------------------------
def build_nc_v2(t_steps: int = T_FULL, repeats: int = 1):
    """Step-parity variant: even/odd steps use disjoint physical PSUM banks,
    so step t's Tanh (ScalarE read) overlaps step t+1's matmuls (PE write)
    without the fatal-collision serialization.  Each block's 8 (parity, m)
    chunks are 256-col half-banks => a block is 4 banks, double-buffered.
    Requires xt with each 32-step block's columns parity-grouped."""
    _patch_drain()
    assert t_steps % TC == 0
    nblk = t_steps // TC
    HC = TC // 2  # steps per parity per block
    nc = bass.Bass("TRN2", target_bir_lowering=False, debug=False)

    xt = nc.declare_dram_parameter("xt", [KC, 128, t_steps, BL], BF16, isOutput=False)
    whh = nc.declare_dram_parameter("whh", [128, KC, MC, 128], BF16, isOutput=False)
    wxh = nc.declare_dram_parameter("wxh", [128, KC, MC, 128], BF16, isOutput=False)
    bh = nc.declare_dram_parameter("bh", [1, MC, 128], BF16, isOutput=False)
    hs = nc.declare_dram_parameter("hs", [MC, 128, t_steps, BL], BF16, isOutput=True)

    xt_perm = xt[:, :, :, :].rearrange("k p t b -> p k t b")
    hs_perm = hs[:, :, :, :].rearrange("m p t b -> p m t b")

    with tile.TileContext(nc) as tc:
        with (
            tc.tile_pool(name="singles", bufs=1) as singles,
            tc.tile_pool(name="xtp", bufs=2) as xt_pool,
            tc.tile_pool(name="histp", bufs=3) as hist_pool,
            tc.tile_pool(name="psump", bufs=2, space="PSUM") as psum_pool,
        ):
            whh_sb = singles.tile([128, KC, MC, 128], BF16)
            nc.sync.dma_start(out=whh_sb, in_=whh[:, :, :, :])
            wxh_sb = singles.tile([128, KC, MC, 128], BF16)
            nc.sync.dma_start(out=wxh_sb, in_=wxh[:, :, :, :])
            bh_sb = singles.tile([1, MC, 128], BF16)
            nc.sync.dma_start(out=bh_sb, in_=bh[:, :, :])
            ones_sb = singles.tile([1, 256], BF16)
            nc.vector.memset(ones_sb, 1.0)

            state = {"prev_hist": None}

            def emit_rec(ps, blk, pass_first):
                # ps: [128, 8, 256]; chunk c = (tl%2)*4 + m, col = (tl//2)*16
                hist = hist_pool.tile([128, MC, TC, BL], BF16, tag="hist")
                for tl in range(TC):
                    par = tl % 2
                    col = (tl // 2) * BL
                    if not (pass_first and tl == 0):
                        src = state["prev_hist"] if tl == 0 else hist
                        pt = TC - 1 if tl == 0 else tl - 1
                        for k in range(KC):
                            for m in range(MC):
                                nc.tensor.matmul(
                                    ps[:, par * 4 + m, col:col + BL],
                                    lhsT=whh_sb[:, k, m, :],
                                    rhs=src[:, k, pt, :],
                                    start=False,
                                    stop=(k == KC - 1),
                                    skip_group_check=True,
                                )
                    nc.scalar.activation(
                        hist[:, 0:2, tl, :],
                        ps[:, par * 4:par * 4 + 2, col:col + BL],
                        mybir.ActivationFunctionType.Tanh,
                    )
                    nc.scalar.activation(
                        hist[:, 2:4, tl, :],
                        ps[:, par * 4 + 2:par * 4 + 4, col:col + BL],
                        mybir.ActivationFunctionType.Tanh,
                    )
                nc.sync.dma_start(
                    out=hs_perm[:, :, blk * TC:(blk + 1) * TC, :], in_=hist
                )
                state["prev_hist"] = hist

            pending = None
            for rep in range(repeats):
                for blk in range(nblk):
                    xt_sb = xt_pool.tile([128, KC, 2, HC * BL], BF16, tag="xt")
                    nc.sync.dma_start(
                        out=xt_sb,
                        in_=xt_perm[:, :, blk * TC:(blk + 1) * TC, :],
                    )
                    ps = psum_pool.tile([128, 8, HC * BL], F32, tag="ps")
                    for par in range(2):
                        for m in range(MC):
                            c = par * 4 + m
                            for k in range(KC):
                                # start=True only on the first MM touching a
                                # physical bank (chunks 2i/2i+1 share a bank):
                                # it clears the whole bank; the sibling
                                # chunk's first MM then overwrites-on-clear.
                                nc.tensor.matmul(
                                    ps[:, c, :],
                                    lhsT=wxh_sb[:, k, m, :],
                                    rhs=xt_sb[:, k, par, :],
                                    start=(k == 0 and m % 2 == 0),
                                    stop=False,
                                    skip_group_check=True,
                                )
                            nc.tensor.matmul(
                                ps[:, c, :],
                                lhsT=bh_sb[:, m, :],
                                rhs=ones_sb[:, :],
                                start=False,
                                stop=True,
                                skip_group_check=True,
                            )
                    if pending is not None:
                        emit_rec(*pending)
                    pending = (ps, blk, blk == 0)
                emit_rec(*pending)
                pending = None

    _split_waits(nc)
    return nc


USE_V2 = False

_nc_cache = {}


def get_nc(t_steps: int = T_FULL):
    if t_steps not in _nc_cache:
        builder = build_nc_v2 if USE_V2 else build_nc
        _nc_cache[t_steps] = builder(t_steps)
    return _nc_cache[t_steps]


def _parity_group_xt(xt_arr, t_steps):
    """Group each 32-step block's time columns by parity:
    [..., T, BL] -> per block order [0,2,...,30, 1,3,...,31]."""
    nblk = t_steps // TC
    a = xt_arr.reshape(KC, 128, nblk, TC // 2, 2, BL)
    a = a.transpose(0, 1, 2, 4, 3, 5)
    return np.ascontiguousarray(a).reshape(KC, 128, t_steps, BL)


def kernel(inputs, W_xh_f, W_hh_f, b_h_f, W_xh_b, W_hh_b, b_h_b):
    nc = get_nc(T_FULL)
    in_maps = make_in_maps(
        inputs, W_xh_f, W_hh_f, b_h_f, W_xh_b, W_hh_b, b_h_b, T_FULL
    )
    res = run_bass_kernel_spmd(nc, in_maps, list(range(N_CORES)))
    return assemble_outputs(res.results, T_FULL)
